# revision 2
# baseline (speedup 1.0000x reference)
"""Dense transformer layer (B2 S2048 D1024 H16) on 8 Trainium2 cores, v2.

Sharding: core c handles batch c//4, query rows (c%4)*512.
K/V projections are computed only for the core's own 512 rows, then
AllGather'd across each 4-core batch group (collective overlaps Q proj).
All matmul operands are bf16 (fp32 accumulation in PSUM); LN stats fp32.
"""
from contextlib import ExitStack

import concourse.bass as bass
import concourse.mybir as mybir
import concourse.tile as tile
from concourse import bacc

F32 = mybir.dt.float32
BF16 = mybir.dt.bfloat16
AF = mybir.ActivationFunctionType
ALU = mybir.AluOpType
LN_EPS = 1e-5


def build(S=2048, SQ=512, D=1024, H=16, DFF=4096, n_cores=8,
          compile=True, pt_bufs=4, sc_bufs=2, kth_bufs=2,
          wp_bufs=8, wfp_bufs=8, drain_bufs=3, fdr_bufs=3, sq_bufs=3,
          wop_bufs=3, vps_bufs=8, fps_bufs=8, reps=1, kv_shard=True,
          skip=()):
    P = 128
    HD = D // H                      # 64
    DC = D // P                      # 8 feature chunks
    FC = DFF // P                    # 32 ffn chunks
    SC = S // P                      # 16 key-row chunks
    NQ = SQ                          # q free dim (<=512)
    NG = n_cores // 2                # batch-group size (4)
    assert NQ <= 512
    RG = [[g * NG + i for i in range(NG)] for g in range(n_cores // NG)]

    nc = bacc.Bacc("TRN2", target_bir_lowering=False, num_devices=n_cores)

    xqT = nc.dram_tensor("xqT", [D, SQ], BF16, kind="ExternalInput")
    xT = (nc.dram_tensor("xT", [D, S], BF16, kind="ExternalInput")
          if not kv_shard else None)
    W = {}
    for name, shp in [("Wq", [D, D]), ("Wk", [D, D]), ("Wv", [D, D]),
                      ("Wo", [D, D]), ("W1", [D, DFF]), ("W2", [DFF, D])]:
        W[name] = nc.dram_tensor(name, shp, BF16, kind="ExternalInput")
    vecs = {}
    for name, n in [("bqs", D), ("bk", D), ("bv", D), ("bo", D), ("bf1", DFF),
                    ("bf2", D), ("g1", D), ("b1n", D), ("g2", D), ("b2n", D)]:
        vecs[name] = nc.dram_tensor(name, [n], F32, kind="ExternalInput")
    ones_d = nc.dram_tensor("ones_d", [P], BF16, kind="ExternalInput")
    yT = nc.dram_tensor("yT", [D, SQ], F32, kind="ExternalOutput")

    with tile.TileContext(nc) as tc, ExitStack() as top:
        const = top.enter_context(tc.tile_pool(name="const", bufs=1))
        dram = top.enter_context(tc.tile_pool(name="dram", bufs=1, space="DRAM"))

        # per-feature vectors as [P, n/P] tiles (feature f -> [f % P, f // P])
        vt = {}
        for name, n in [("bqs", D), ("bk", D), ("bo", D), ("bf1", DFF),
                        ("bf2", D), ("g1", D), ("b1n", D), ("g2", D),
                        ("b2n", D)]:
            t = const.tile([P, n // P], F32, tag=f"vec_{name}")
            nc.sync.dma_start(out=t, in_=vecs[name].ap().rearrange(
                "(t p) -> p t", p=P))
            vt[name] = t
        bvb = const.tile([P, D], F32, tag="bvb")  # bv broadcast over rows
        nc.sync.dma_start(out=bvb, in_=vecs["bv"].ap().partition_broadcast(P))
        ones_col = const.tile([P, 1], BF16, tag="ones_col")
        nc.sync.dma_start(out=ones_col,
                          in_=ones_d.ap()[0:1].partition_broadcast(P))
        ones_row = const.tile([1, P], BF16, tag="ones_row")
        nc.sync.dma_start(out=ones_row, in_=ones_d.ap()[0:P])
        eps_t = const.tile([1, 1], F32, tag="eps_t")
        nc.vector.memset(eps_t, LN_EPS)

        # DRAM scratch: own K/V chunks + gathered full K/V.
        # K and V are each split into two half-feature collectives so the
        # first AllGather leaves as soon as half the projection is drained.
        DH = D // 2
        if kv_shard:
            kt_in = [dram.tile([DH, SQ], BF16, tag=f"kt_in{i}",
                               name=f"kt_in{i}") for i in range(2)]
            v_in = [dram.tile([SQ, DH], BF16, tag=f"v_in{i}",
                              name=f"v_in{i}") for i in range(2)]
            kt_all = [dram.tile([NG * DH, SQ], BF16, tag=f"kt_all{i}",
                                name=f"kt_all{i}") for i in range(2)]
            v_all = [dram.tile([S, DH], BF16, tag=f"v_all{i}",
                               name=f"v_all{i}") for i in range(2)]
        else:
            kt_full = dram.tile([D, S], BF16, tag="kt_full")

        for _rep in range(reps):
            # pools that cross phase boundaries (midp before actp: LIFO order)
            midp_cm = tc.tile_pool(name="midp", bufs=1)
            midp = midp_cm.__enter__()
            ln_in = midp.tile([P, DC, NQ], BF16, tag="ln_in")
            actp_cm = tc.tile_pool(name="actp", bufs=1)   # qt, xq, v_sb
            actp = actp_cm.__enter__()
            qt = actp.tile([P, DC, NQ], BF16, tag="qt")    # QT (Wq pre-scaled)
            xq = actp.tile([P, DC, NQ], BF16, tag="xq")
            v_sb = actp.tile([P, SC, D], BF16, tag="v_sb")

            # ================= phase 1: projections =================
            with ExitStack() as ph:
                p1 = ph.enter_context(tc.tile_pool(name="p1", bufs=1))
                wp = ph.enter_context(tc.tile_pool(name="wp", bufs=wp_bufs))
                drain = ph.enter_context(tc.tile_pool(name="drain",
                                                      bufs=drain_bufs))
                psum = ph.enter_context(tc.tile_pool(name="ps1", bufs=vps_bufs,
                                                     space="PSUM"))

                for c in range(DC):
                    nc.sync.dma_start(out=xq[:, c, :],
                                      in_=xqT[c * P:(c + 1) * P, :])

                if kv_shard:
                    # ---- K for own rows -> kt_in -> AllGather ----
                    pss = [psum.tile([P, NQ], F32, tag="pp", name=f"pp_k{i}")
                           for i in range(DC)]
                    for c in range(DC):
                        wt = wp.tile([P, D], BF16, tag="wt", name=f"wkt{c}")
                        nc.sync.dma_start(out=wt,
                                          in_=W["Wk"][c * P:(c + 1) * P, :])
                        for t in range(DC):
                            nc.tensor.matmul(
                                pss[t], wt[:, t * P:(t + 1) * P], xq[:, c, :],
                                start=(c == 0), stop=(c == DC - 1))
                    for half in range(2):
                        for t4 in range(DC // 2):
                            t = half * (DC // 2) + t4
                            kd = drain.tile([P, NQ], BF16, tag="kd",
                                            name=f"kd{t}")
                            nc.vector.tensor_scalar(
                                out=kd, in0=pss[t],
                                scalar1=vt["bk"][:, t:t + 1],
                                scalar2=None, op0=ALU.add)
                            nc.sync.dma_start(
                                out=kt_in[half][t4 * P:(t4 + 1) * P, :],
                                in_=kd)
                        nc.gpsimd.collective_compute(
                            "AllGather", ALU.bypass, replica_groups=RG,
                            ins=[kt_in[half][:, :].opt()],
                            outs=[kt_all[half][:, :].opt()])

                    # ---- V for own rows (natural) -> v_in -> AllGather ----
                    DB = 512
                    for dv in range(D // DB):
                        pssv = [psum.tile([P, DB], F32, tag="pp",
                                          name=f"pp_v{dv}_{i}")
                                for i in range(SQ // P)]
                        for c in range(DC):
                            wt = wp.tile([P, DB], BF16, tag="wt",
                                         name=f"wvt{dv}_{c}")
                            nc.sync.dma_start(
                                out=wt,
                                in_=W["Wv"][c * P:(c + 1) * P,
                                            dv * DB:(dv + 1) * DB])
                            for rb in range(SQ // P):
                                nc.tensor.matmul(
                                    pssv[rb], xq[:, c, rb * P:(rb + 1) * P],
                                    wt, start=(c == 0), stop=(c == DC - 1))
                        for rb in range(SQ // P):
                            vd = drain.tile([P, DB], BF16, tag="vd",
                                            name=f"vd{dv}_{rb}")
                            nc.vector.tensor_add(
                                vd, pssv[rb], bvb[:, dv * DB:(dv + 1) * DB])
                            nc.sync.dma_start(
                                out=v_in[dv][rb * P:(rb + 1) * P, :],
                                in_=vd)
                        nc.gpsimd.collective_compute(
                            "AllGather", ALU.bypass, replica_groups=RG,
                            ins=[v_in[dv][:, :].opt()],
                            outs=[v_all[dv][:, :].opt()])
                else:
                    # ---- redundant full-row K/V (no collectives) ----
                    SH = S // 2
                    SCH = SH // P
                    for half in range(2):
                        xtc = [p1.tile([P, SH], BF16, tag=f"xt{c}",
                                       name=f"xt{half}_{c}")
                               for c in range(DC)]
                        for c in range(DC):
                            nc.sync.dma_start(
                                out=xtc[c],
                                in_=xT[c * P:(c + 1) * P,
                                       half * SH:(half + 1) * SH])
                        for th in range(2):
                            pss = [psum.tile([P, SH // 2], F32, tag="pp",
                                             name=f"ppk{half}_{th}_{i}")
                                   for i in range(8)]
                            for c in range(DC):
                                wt = wp.tile([P, 4 * P], BF16, tag="wt",
                                             name=f"wkt{half}_{th}_{c}")
                                nc.sync.dma_start(
                                    out=wt,
                                    in_=W["Wk"][c * P:(c + 1) * P,
                                                th * 4 * P:(th + 1) * 4 * P])
                                for t4 in range(4):
                                    for rb in range(2):
                                        nc.tensor.matmul(
                                            pss[t4 * 2 + rb],
                                            wt[:, t4 * P:(t4 + 1) * P],
                                            xtc[c][:, rb * (SH // 2):
                                                   (rb + 1) * (SH // 2)],
                                            start=(c == 0), stop=(c == DC - 1))
                            for t4 in range(4):
                                t = th * 4 + t4
                                for rb in range(2):
                                    kd = drain.tile([P, SH // 2], BF16,
                                                    tag="kd",
                                                    name=f"kd{half}_{th}_{t4}_{rb}")
                                    nc.vector.tensor_scalar(
                                        out=kd, in0=pss[t4 * 2 + rb],
                                        scalar1=vt["bk"][:, t:t + 1],
                                        scalar2=None, op0=ALU.add)
                                    nc.sync.dma_start(
                                        out=kt_full[t * P:(t + 1) * P,
                                                    half * SH + rb * (SH // 2):
                                                    half * SH + (rb + 1) * (SH // 2)],
                                        in_=kd)
                        DB = 512
                        for dv in range(D // DB):
                            pss = [psum.tile([P, DB], F32, tag="pp",
                                             name=f"ppv{half}_{dv}_{i}")
                                   for i in range(SCH)]
                            for c in range(DC):
                                wt = wp.tile([P, DB], BF16, tag="wt",
                                             name=f"wvt{half}_{dv}_{c}")
                                nc.sync.dma_start(
                                    out=wt,
                                    in_=W["Wv"][c * P:(c + 1) * P,
                                                dv * DB:(dv + 1) * DB])
                                for r8 in range(SCH):
                                    nc.tensor.matmul(
                                        pss[r8],
                                        xtc[c][:, r8 * P:(r8 + 1) * P],
                                        wt, start=(c == 0), stop=(c == DC - 1))
                            for r8 in range(SCH):
                                rt = half * SCH + r8
                                nc.vector.tensor_add(
                                    v_sb[:, rt, dv * DB:(dv + 1) * DB],
                                    pss[r8],
                                    bvb[:, dv * DB:(dv + 1) * DB])

                # ---- Q for own rows (after K/V so collectives start early) --
                pss = [psum.tile([P, NQ], F32, tag="pp", name=f"pp_q{i}")
                       for i in range(DC)]
                for c in range(DC):
                    wt = wp.tile([P, D], BF16, tag="wt", name=f"wqt{c}")
                    nc.sync.dma_start(out=wt, in_=W["Wq"][c * P:(c + 1) * P, :])
                    for t in range(DC):
                        nc.tensor.matmul(
                            pss[t], wt[:, t * P:(t + 1) * P], xq[:, c, :],
                            start=(c == 0), stop=(c == DC - 1))
                for t in range(DC):
                    nc.vector.tensor_scalar(
                        out=qt[:, t, :], in0=pss[t],
                        scalar1=vt["bqs"][:, t:t + 1], scalar2=None,
                        op0=ALU.add)

            # ================= phase 2: attention =================
            attnp_cm = tc.tile_pool(name="attnp", bufs=1)
            attnp = attnp_cm.__enter__()
            # paired layout: even head on partitions 0-63, odd head on 64-127
            attn_t2 = attnp.tile([P, H // 2, NQ], BF16, tag="attn_t2")
            if kv_shard:
                for dv in range(2):
                    for sc in range(SC):
                        nc.sync.dma_start(
                            out=v_sb[:, sc, dv * DH:(dv + 1) * DH],
                            in_=v_all[dv][sc * P:(sc + 1) * P, :])
            # prefetch Wo (head-pair-stacked rows) during attention
            wop_cm = tc.tile_pool(name="wop", bufs=H // 2)
            wop = wop_cm.__enter__()
            wo_t = []
            for p8 in range(H // 2):
                wt = wop.tile([P, D], BF16, tag="wot", name=f"wot{p8}")
                nc.sync.dma_start(out=wt,
                                  in_=W["Wo"][p8 * P:(p8 + 1) * P, :])
                wo_t.append(wt)
            with ExitStack() as ph:
                kth_p = ph.enter_context(tc.tile_pool(name="kth",
                                                      bufs=kth_bufs))
                pt_p = ph.enter_context(tc.tile_pool(name="pt", bufs=pt_bufs))
                nrm = ph.enter_context(tc.tile_pool(name="nrm", bufs=2))
                sc_ps = ph.enter_context(tc.tile_pool(name="scp", bufs=sc_bufs,
                                                      space="PSUM"))
                at_ps = ph.enter_context(tc.tile_pool(name="atp", bufs=1,
                                                      space="PSUM"))
                dn_ps = ph.enter_context(tc.tile_pool(name="dnp", bufs=2,
                                                      space="PSUM"))
                bc_ps = ph.enter_context(tc.tile_pool(name="bcp", bufs=1,
                                                      space="PSUM"))

                def load_kth(hp):
                    h0, h1 = 2 * hp, 2 * hp + 1
                    kth = kth_p.tile([P, S], BF16, tag="kth",
                                     name=f"kth{hp}")
                    if kv_shard:
                        kh, f0 = divmod(h0 * HD, DH)
                        f1 = (h1 * HD) % DH
                        for r in range(NG):
                            nc.sync.dma_start(
                                out=kth[0:HD, r * SQ:(r + 1) * SQ],
                                in_=kt_all[kh][r * DH + f0:
                                               r * DH + f0 + HD, :])
                            nc.sync.dma_start(
                                out=kth[HD:2 * HD, r * SQ:(r + 1) * SQ],
                                in_=kt_all[kh][r * DH + f1:
                                               r * DH + f1 + HD, :])
                    else:
                        nc.sync.dma_start(out=kth[0:HD, :],
                                          in_=kt_full[h0 * HD:(h0 + 1) * HD, :])
                        nc.sync.dma_start(out=kth[HD:2 * HD, :],
                                          in_=kt_full[h1 * HD:(h1 + 1) * HD, :])
                    return kth

                def make_scores(hp, cp, kth):
                    sps = [sc_ps.tile([P, 2, NQ], F32, tag="sp",
                                      name=f"sp{hp}_{cp}_{j}")
                           for j in range(2)]
                    for j in range(2):
                        ck = cp * 2 + j
                        for hh, pol in ((0, 0), (1, HD)):
                            nc.tensor.matmul(
                                sps[hh][:, j, :],
                                kth[pol:pol + HD, ck * P:(ck + 1) * P],
                                qt[pol:pol + HD, hp, :],
                                start=True, stop=True)
                    return sps

                def normalize(hp, aps, dn):
                    rcps = []
                    for hh, dpart in ((0, 0), (1, 32)):
                        rcp = nrm.tile([1, NQ], BF16, tag="rcp",
                                       name=f"rcp{hp}_{hh}")
                        with nc.allow_low_precision(reason="bf16 bcast feed"):
                            nc.vector.reciprocal(out=rcp,
                                                 in_=dn[dpart:dpart + 1, :])
                        rcps.append(rcp)
                    bcp = bc_ps.tile([P, NQ], F32, tag="bcp",
                                     name=f"bcp{hp}")
                    nc.tensor.matmul(bcp[0:HD, :], ones_row[:, 0:HD], rcps[0],
                                     start=True, stop=True,
                                     tile_position=(0, 0))
                    nc.tensor.matmul(bcp[HD:P, :], ones_row[:, 0:HD], rcps[1],
                                     start=True, stop=True,
                                     tile_position=(0, 64))
                    bcs = nrm.tile([P, NQ], F32, tag="bcs", name=f"bcs{hp}")
                    nc.vector.tensor_copy(out=bcs, in_=bcp)
                    nc.vector.tensor_mul(attn_t2[:, hp, :], aps, bcs)

                # software-pipelined: scores(cp+1) issues ahead of attnV(cp)
                # so the in-order PE queue never blocks the ACT exp stream;
                # the previous pair's normalization is folded into this
                # pair's first iteration.
                pend = None   # (hp, aps, dn) awaiting normalization
                NPAIR = H // 2 if "attn" not in skip else 0
                kth_cur = load_kth(0) if NPAIR else None
                for hp in range(NPAIR):
                    h0, h1 = 2 * hp, 2 * hp + 1
                    aps = at_ps.tile([P, NQ], F32, tag="aps",
                                     name=f"aps{hp}")
                    dn = dn_ps.tile([33, NQ], F32, tag="dn", name=f"dn{hp}")
                    s_cur = make_scores(hp, 0, kth_cur)
                    kth_nxt = load_kth(hp + 1) if hp + 1 < NPAIR else None
                    for cp in range(SC // 2):
                        pts = []
                        for hh in range(2):
                            pt = pt_p.tile([P, 2, NQ], BF16, tag="pt",
                                           name=f"pt{hp}_{cp}_{hh}")
                            nc.scalar.activation(out=pt, in_=s_cur[hh],
                                                 func=AF.Exp)
                            pts.append(pt)
                        if cp == 0 and pend is not None:
                            # prev pair's normalization: must precede this
                            # pair's attnV in the PE queue (aps slot WAR)
                            normalize(*pend)
                            pend = None
                        if cp + 1 < SC // 2:
                            s_cur = make_scores(hp, cp + 1, kth_cur)
                        for j in range(2):
                            ck = cp * 2 + j
                            # h0 -> psum partitions 0-63, h1 -> 64-127
                            # (concurrent col groups); denominators to dn
                            nc.tensor.matmul(
                                aps[0:HD, :],
                                v_sb[:, ck, h0 * HD:(h0 + 1) * HD],
                                pts[0][:, j, :],
                                start=(ck == 0), stop=(ck == SC - 1),
                                tile_position=(0, 0))
                            nc.tensor.matmul(
                                aps[HD:P, :],
                                v_sb[:, ck, h1 * HD:(h1 + 1) * HD],
                                pts[1][:, j, :],
                                start=(ck == 0), stop=(ck == SC - 1),
                                tile_position=(0, 64))
                            nc.tensor.matmul(
                                dn[0:1, :], ones_col, pts[0][:, j, :],
                                start=(ck == 0), stop=(ck == SC - 1),
                                tile_position=(0, 0))
                            nc.tensor.matmul(
                                dn[32:33, :], ones_col, pts[1][:, j, :],
                                start=(ck == 0), stop=(ck == SC - 1),
                                tile_position=(0, 32))
                    pend = (hp, aps, dn)
                    kth_cur = kth_nxt
                if pend is not None:
                    normalize(*pend)

            # ---- out-proj (head pair stacked: K=128) + residual -> ln_in ---
            with ExitStack() as ph:
                odr = ph.enter_context(tc.tile_pool(name="odr", bufs=3))
                op_ps = ph.enter_context(tc.tile_pool(name="opp", bufs=8,
                                                      space="PSUM"))
                pss = [op_ps.tile([P, NQ], F32, tag="op", name=f"op_{i}")
                       for i in range(DC)]
                for p8 in range(H // 2):
                    for t in range(DC):
                        nc.tensor.matmul(
                            pss[t], wo_t[p8][:, t * P:(t + 1) * P],
                            attn_t2[:, p8, :],
                            start=(p8 == 0), stop=(p8 == H // 2 - 1))
                for t in range(DC):
                    oa = odr.tile([P, NQ], F32, tag="oa")
                    nc.vector.tensor_scalar(
                        out=oa, in0=pss[t], scalar1=vt["bo"][:, t:t + 1],
                        scalar2=None, op0=ALU.add)
                    nc.vector.tensor_add(ln_in[:, t, :], oa, xq[:, t, :])
            wop_cm.__exit__(None, None, None)  # free Wo tiles

            attnp_cm.__exit__(None, None, None)  # free attn_t2
            actp_cm.__exit__(None, None, None)   # free qt, xq, v_sb

            lnp_cm = tc.tile_pool(name="lnp", bufs=1)
            lnp = lnp_cm.__enter__()

            def layer_norm(ph, src, g, b, tag, out_dtype=BF16):
                """src [P, DC, NQ] bf16 feature-major -> LN'd tile from lnp."""
                st_ps = ph.enter_context(tc.tile_pool(name=f"st{tag}", bufs=1,
                                                      space="PSUM"))
                bb_ps = ph.enter_context(tc.tile_pool(name=f"bb{tag}", bufs=2,
                                                      space="PSUM"))
                sqp = ph.enter_context(tc.tile_pool(name=f"sq{tag}",
                                                    bufs=sq_bufs))
                row = ph.enter_context(tc.tile_pool(name=f"row{tag}", bufs=1))

                sm = st_ps.tile([1, NQ], F32, tag="sm")
                sq = st_ps.tile([1, NQ], F32, tag="sq")
                for c in range(DC):
                    nc.tensor.matmul(sm, ones_col, src[:, c, :],
                                     start=(c == 0), stop=(c == DC - 1))
                for c in range(DC):
                    x2 = sqp.tile([P, NQ], BF16, tag="x2")
                    # squares on ACT (idle here); sums run on PE
                    nc.scalar.activation(out=x2, in_=src[:, c, :],
                                         func=AF.Square)
                    nc.tensor.matmul(sq, ones_col, x2,
                                     start=(c == 0), stop=(c == DC - 1))
                mean = row.tile([1, NQ], F32, tag="mean")
                nc.scalar.mul(out=mean, in_=sm, mul=1.0 / D)
                msq = row.tile([1, NQ], F32, tag="msq")
                nc.scalar.mul(out=msq, in_=sq, mul=1.0 / D)
                var = row.tile([1, NQ], F32, tag="var")
                nc.vector.tensor_mul(var, mean, mean)
                nc.vector.tensor_sub(var, msq, var)
                sd = row.tile([1, NQ], F32, tag="sd")
                nc.scalar.activation(out=sd, in_=var, func=AF.Sqrt, bias=eps_t)
                rstd = row.tile([1, NQ], BF16, tag="rstd")
                with nc.allow_low_precision(reason="bf16 feed for bcast"):
                    nc.vector.reciprocal(out=rstd, in_=sd)
                shift = row.tile([1, NQ], BF16, tag="shift")   # -mean*rstd
                nc.vector.tensor_mul(shift, mean, rstd)
                nc.scalar.mul(out=shift, in_=shift, mul=-1.0)

                ab = bb_ps.tile([P, NQ], F32, tag="ab")
                nc.tensor.matmul(ab, ones_row, rstd, start=True, stop=True)
                a_b = sqp.tile([P, NQ], F32, tag="a_b")
                nc.vector.tensor_copy(out=a_b, in_=ab)
                bb = bb_ps.tile([P, NQ], F32, tag="ab")
                nc.tensor.matmul(bb, ones_row, shift, start=True, stop=True)
                b_b = sqp.tile([P, NQ], F32, tag="b_b")
                nc.vector.tensor_copy(out=b_b, in_=bb)

                out_t = lnp.tile([P, DC, NQ], out_dtype, tag=f"ln{tag}")
                for c in range(DC):
                    tmp = sqp.tile([P, NQ], F32, tag="tmp")
                    nc.vector.tensor_mul(tmp, src[:, c, :], a_b)
                    nc.vector.tensor_add(tmp, tmp, b_b)
                    # gamma/beta via ACT (per-partition scale+bias), off DVE
                    nc.scalar.activation(
                        out=out_t[:, c, :], in_=tmp, func=AF.Identity,
                        scale=g[:, c:c + 1], bias=b[:, c:c + 1])
                return out_t

            # ================= phase 3: LN1, FFN, LN2 =================
            with ExitStack() as ph:
                ln1 = layer_norm(ph, ln_in, vt["g1"], vt["b1n"], "1")

            with ExitStack() as ph:
                ffn = ph.enter_context(tc.tile_pool(name="ffn", bufs=1))
                wfp = ph.enter_context(tc.tile_pool(name="wfp", bufs=wfp_bufs))
                fdr = ph.enter_context(tc.tile_pool(name="fdr", bufs=fdr_bufs))
                f_ps = ph.enter_context(tc.tile_pool(name="fps", bufs=fps_bufs,
                                                     space="PSUM"))
                h1 = ffn.tile([P, FC, NQ], BF16, tag="h1")
                for pg in (() if "ffn" in skip else range(FC // 8)):
                    pss = [f_ps.tile([P, NQ], F32, tag="fp",
                                     name=f"fp1_{pg}_{i}") for i in range(8)]
                    for c in range(DC):
                        wt = wfp.tile([P, 8 * P], BF16, tag="w1t")
                        nc.sync.dma_start(
                            out=wt,
                            in_=W["W1"][c * P:(c + 1) * P,
                                        pg * 8 * P:(pg + 1) * 8 * P])
                        for t8 in range(8):
                            nc.tensor.matmul(
                                pss[t8], wt[:, t8 * P:(t8 + 1) * P],
                                ln1[:, c, :],
                                start=(c == 0), stop=(c == DC - 1))
                    for t8 in range(8):
                        t = pg * 8 + t8
                        nc.scalar.activation(
                            out=h1[:, t, :], in_=pss[t8], func=AF.Gelu,
                            bias=vt["bf1"][:, t:t + 1])

                ln2_in = midp.tile([P, DC, NQ], BF16, tag="ln2_in")
                if "ffn" in skip:
                    for t in range(DC):
                        nc.vector.tensor_copy(out=ln2_in[:, t, :],
                                              in_=ln1[:, t, :])
                else:
                    pss = [f_ps.tile([P, NQ], F32, tag="fp", name=f"fp2_{i}")
                           for i in range(DC)]
                    for c in range(FC):
                        wt = wfp.tile([P, D], BF16, tag="w2t")
                        nc.sync.dma_start(out=wt,
                                          in_=W["W2"][c * P:(c + 1) * P, :])
                        for t in range(DC):
                            nc.tensor.matmul(
                                pss[t], wt[:, t * P:(t + 1) * P],
                                h1[:, c, :],
                                start=(c == 0), stop=(c == FC - 1))
                    for t in range(DC):
                        fo = fdr.tile([P, NQ], F32, tag="fo")
                        nc.vector.tensor_scalar(
                            out=fo, in0=pss[t], scalar1=vt["bf2"][:, t:t + 1],
                            scalar2=None, op0=ALU.add)
                        nc.vector.tensor_add(ln2_in[:, t, :], fo, ln1[:, t, :])

            with ExitStack() as ph:
                y_out = layer_norm(ph, ln2_in, vt["g2"], vt["b2n"], "2",
                                   out_dtype=F32)
                yv = yT.ap().rearrange("(t p) q -> p t q", p=P)
                for c in range(DC):
                    nc.sync.dma_start(out=yv[:, c, :], in_=y_out[:, c, :])
            lnp_cm.__exit__(None, None, None)
            midp_cm.__exit__(None, None, None)

    if compile:
        nc.compile()
    return nc


# ---------------- host-side sharding / gather ----------------
import numpy as np
import ml_dtypes

BF16_NP = ml_dtypes.bfloat16
B, S, D, H = 2, 2048, 1024, 16
HD = D // H
DFF = 4 * D
N_CORES = 8
CPB = N_CORES // B           # cores per batch element
SQ = S // CPB                # query rows per core

_nc = None


def _get_nc():
    global _nc
    if _nc is None:
        _nc = build(S=S, SQ=SQ, D=D, H=H, DFF=DFF, n_cores=N_CORES)
    return _nc


def _make_in_maps(inputs):
    x = np.ascontiguousarray(inputs["x"], dtype=np.float32)
    scale = np.float32(1.0 / np.sqrt(HD))
    bf = lambda a: np.ascontiguousarray(np.asarray(a, np.float32)).astype(BF16_NP)
    f32 = lambda a: np.ascontiguousarray(inputs[a], np.float32)
    shared = {
        "Wq": bf(np.asarray(inputs["Wq"], np.float32) * scale),
        "Wk": bf(inputs["Wk"]),
        "Wv": bf(inputs["Wv"]),
        "Wo": bf(inputs["Wo"]),
        "W1": bf(inputs["W1"]),
        "W2": bf(inputs["W2"]),
        "bqs": np.ascontiguousarray(inputs["bq"], np.float32) * scale,
        "bk": f32("bk"), "bv": f32("bv"), "bo": f32("bo"),
        "bf1": f32("bf1"), "bf2": f32("bf2"),
        "g1": f32("g1"), "b1n": f32("b1n"), "g2": f32("g2"), "b2n": f32("b2n"),
        "ones_d": np.ones(128, BF16_NP),
    }
    xT = np.ascontiguousarray(x.transpose(0, 2, 1)).astype(BF16_NP)  # [B,D,S]
    in_maps = []
    for c in range(N_CORES):
        b, q0 = c // CPB, (c % CPB) * SQ
        m = dict(shared)
        m["xqT"] = np.ascontiguousarray(xT[b][:, q0:q0 + SQ])
        in_maps.append(m)
    return in_maps


def kernel(**inputs):
    from concourse.bass_utils import run_bass_kernel_spmd
    nc = _get_nc()
    in_maps = _make_in_maps(inputs)
    res = run_bass_kernel_spmd(nc, in_maps, core_ids=list(range(N_CORES)))
    y = np.empty((B, S, D), dtype=np.float32)
    for c in range(N_CORES):
        b, q0 = c // CPB, (c % CPB) * SQ
        y[b, q0:q0 + SQ, :] = res.results[c]["yT"].T
    return y


# revision 3
# speedup vs baseline: 1.3280x; 1.3280x over previous
"""Dense transformer layer (B2 S2048 D1024 H16) on 8 Trainium2 cores, v2.

Sharding: core c handles batch c//4, query rows (c%4)*512.
K/V projections are computed only for the core's own 512 rows, then
AllGather'd across each 4-core batch group (collective overlaps Q proj).
All matmul operands are bf16 (fp32 accumulation in PSUM); LN stats fp32.
"""
from contextlib import ExitStack

import concourse.bass as bass
import concourse.mybir as mybir
import concourse.tile as tile
from concourse import bacc

F32 = mybir.dt.float32
BF16 = mybir.dt.bfloat16
AF = mybir.ActivationFunctionType
ALU = mybir.AluOpType
LN_EPS = 1e-5


def build(S=2048, SQ=512, D=1024, H=16, DFF=4096, n_cores=8,
          compile=True, pt_bufs=4, sc_bufs=2, kth_bufs=2,
          wp_bufs=8, wfp_bufs=8, drain_bufs=3, fdr_bufs=3, sq_bufs=3,
          wop_bufs=3, vps_bufs=8, fps_bufs=8, reps=1, kv_shard=True,
          skip=()):
    P = 128
    HD = D // H                      # 64
    DC = D // P                      # 8 feature chunks
    FC = DFF // P                    # 32 ffn chunks
    SC = S // P                      # 16 key-row chunks
    NQ = SQ                          # q free dim (<=512)
    NG = n_cores // 2                # batch-group size (4)
    assert NQ <= 512
    RG = [[g * NG + i for i in range(NG)] for g in range(n_cores // NG)]

    nc = bacc.Bacc("TRN2", target_bir_lowering=False, num_devices=n_cores)

    xqT = nc.dram_tensor("xqT", [D, SQ], BF16, kind="ExternalInput")
    xT = (nc.dram_tensor("xT", [D, S], BF16, kind="ExternalInput")
          if not kv_shard else None)
    W = {}
    for name, shp in [("Wq", [D, D]), ("Wk", [D, D]), ("Wv", [D, D]),
                      ("Wo", [D, D]), ("W1", [D, DFF]), ("W2", [DFF, D])]:
        W[name] = nc.dram_tensor(name, shp, BF16, kind="ExternalInput")
    vecs = {}
    for name, n in [("bqs", D), ("bk", D), ("bv", D), ("bo", D), ("bf1", DFF),
                    ("bf2", D), ("g1", D), ("b1n", D), ("g2", D), ("b2n", D)]:
        vecs[name] = nc.dram_tensor(name, [n], F32, kind="ExternalInput")
    ones_d = nc.dram_tensor("ones_d", [P], BF16, kind="ExternalInput")
    yT = nc.dram_tensor("yT", [D, SQ], F32, kind="ExternalOutput")

    with tile.TileContext(nc) as tc, ExitStack() as top:
        const = top.enter_context(tc.tile_pool(name="const", bufs=1))
        dram = top.enter_context(tc.tile_pool(name="dram", bufs=1, space="DRAM"))

        # per-feature vectors as [P, n/P] tiles (feature f -> [f % P, f // P])
        vt = {}
        for name, n in [("bqs", D), ("bk", D), ("bo", D), ("bf1", DFF),
                        ("bf2", D), ("g1", D), ("b1n", D), ("g2", D),
                        ("b2n", D)]:
            t = const.tile([P, n // P], F32, tag=f"vec_{name}")
            nc.sync.dma_start(out=t, in_=vecs[name].ap().rearrange(
                "(t p) -> p t", p=P))
            vt[name] = t
        bvb = const.tile([P, D], F32, tag="bvb")  # bv broadcast over rows
        nc.sync.dma_start(out=bvb, in_=vecs["bv"].ap().partition_broadcast(P))
        ones_col = const.tile([P, 1], BF16, tag="ones_col")
        nc.sync.dma_start(out=ones_col,
                          in_=ones_d.ap()[0:1].partition_broadcast(P))
        ones_row = const.tile([1, P], BF16, tag="ones_row")
        nc.sync.dma_start(out=ones_row, in_=ones_d.ap()[0:P])
        eps_t = const.tile([1, 1], F32, tag="eps_t")
        nc.vector.memset(eps_t, LN_EPS)

        # DRAM scratch: own K/V chunks + gathered full K/V.
        # K and V are each split into two half-feature collectives so the
        # first AllGather leaves as soon as half the projection is drained.
        DH = D // 2
        if kv_shard:
            kt_in = [dram.tile([DH, SQ], BF16, tag=f"kt_in{i}",
                               name=f"kt_in{i}") for i in range(2)]
            v_in = [dram.tile([SQ, DH], BF16, tag=f"v_in{i}",
                              name=f"v_in{i}") for i in range(2)]
            kt_all = [dram.tile([NG * DH, SQ], BF16, tag=f"kt_all{i}",
                                name=f"kt_all{i}") for i in range(2)]
            v_all = [dram.tile([S, DH], BF16, tag=f"v_all{i}",
                               name=f"v_all{i}") for i in range(2)]
        else:
            kt_full = dram.tile([D, S], BF16, tag="kt_full")

        for _rep in range(reps):
            # pools that cross phase boundaries (midp before actp: LIFO order)
            midp_cm = tc.tile_pool(name="midp", bufs=1)
            midp = midp_cm.__enter__()
            ln_in = midp.tile([P, DC, NQ], BF16, tag="ln_in")
            actp_cm = tc.tile_pool(name="actp", bufs=1)   # qt, xq, v_sb
            actp = actp_cm.__enter__()
            qt = actp.tile([P, DC, NQ], BF16, tag="qt")    # QT (Wq pre-scaled)
            xq = actp.tile([P, DC, NQ], BF16, tag="xq")
            v_sb = actp.tile([P, SC, D], BF16, tag="v_sb")

            # ================= phase 1: projections =================
            with ExitStack() as ph:
                p1 = ph.enter_context(tc.tile_pool(name="p1", bufs=1))
                wp = ph.enter_context(tc.tile_pool(name="wp", bufs=wp_bufs))
                drain = ph.enter_context(tc.tile_pool(name="drain",
                                                      bufs=drain_bufs))
                psum = ph.enter_context(tc.tile_pool(name="ps1", bufs=vps_bufs,
                                                     space="PSUM"))

                for c in range(DC):
                    nc.sync.dma_start(out=xq[:, c, :],
                                      in_=xqT[c * P:(c + 1) * P, :])

                if kv_shard:
                    # K/V output chunks computed t-outer (accumulate over all
                    # c, drain immediately) and interleaved K-a, V-a, K-b, V-b
                    # so the first AllGathers leave as early as possible;
                    # attention pair 0 needs only K-a + V-a.
                    wkt, wvt = [], []
                    for c in range(DC):
                        wt = wp.tile([P, D], BF16, tag="wkt", name=f"wkt{c}")
                        nc.sync.dma_start(out=wt,
                                          in_=W["Wk"][c * P:(c + 1) * P, :])
                        wkt.append(wt)
                    for c in range(DC):
                        wt = wp.tile([P, D], BF16, tag="wvt", name=f"wvt{c}")
                        nc.sync.dma_start(out=wt,
                                          in_=W["Wv"][c * P:(c + 1) * P, :])
                        wvt.append(wt)
                    DB = 512
                    for half in range(2):
                        # ---- K chunks t = half*4 .. half*4+3 ----
                        for t4 in range(DC // 2):
                            t = half * (DC // 2) + t4
                            ps = psum.tile([P, NQ], F32, tag="pp",
                                           name=f"pp_k{t}")
                            for c in range(DC):
                                nc.tensor.matmul(
                                    ps, wkt[c][:, t * P:(t + 1) * P],
                                    xq[:, c, :],
                                    start=(c == 0), stop=(c == DC - 1))
                            kd = drain.tile([P, NQ], BF16, tag="kd",
                                            name=f"kd{t}")
                            nc.vector.tensor_scalar(
                                out=kd, in0=ps,
                                scalar1=vt["bk"][:, t:t + 1],
                                scalar2=None, op0=ALU.add)
                            nc.sync.dma_start(
                                out=kt_in[half][t4 * P:(t4 + 1) * P, :],
                                in_=kd)
                        nc.gpsimd.collective_compute(
                            "AllGather", ALU.bypass, replica_groups=RG,
                            ins=[kt_in[half][:, :].opt()],
                            outs=[kt_all[half][:, :].opt()])
                        # ---- V feature half dv = half (natural layout) ----
                        dv = half
                        pssv = [psum.tile([P, DB], F32, tag="pp",
                                          name=f"pp_v{dv}_{i}")
                                for i in range(SQ // P)]
                        for c in range(DC):
                            for rb in range(SQ // P):
                                nc.tensor.matmul(
                                    pssv[rb], xq[:, c, rb * P:(rb + 1) * P],
                                    wvt[c][:, dv * DB:(dv + 1) * DB],
                                    start=(c == 0), stop=(c == DC - 1))
                        for rb in range(SQ // P):
                            vd = drain.tile([P, DB], BF16, tag="vd",
                                            name=f"vd{dv}_{rb}")
                            nc.vector.tensor_add(
                                vd, pssv[rb], bvb[:, dv * DB:(dv + 1) * DB])
                            nc.sync.dma_start(
                                out=v_in[dv][rb * P:(rb + 1) * P, :],
                                in_=vd)
                        nc.gpsimd.collective_compute(
                            "AllGather", ALU.bypass, replica_groups=RG,
                            ins=[v_in[dv][:, :].opt()],
                            outs=[v_all[dv][:, :].opt()])
                else:
                    # ---- redundant full-row K/V (no collectives) ----
                    SH = S // 2
                    SCH = SH // P
                    for half in range(2):
                        xtc = [p1.tile([P, SH], BF16, tag=f"xt{c}",
                                       name=f"xt{half}_{c}")
                               for c in range(DC)]
                        for c in range(DC):
                            nc.sync.dma_start(
                                out=xtc[c],
                                in_=xT[c * P:(c + 1) * P,
                                       half * SH:(half + 1) * SH])
                        for th in range(2):
                            pss = [psum.tile([P, SH // 2], F32, tag="pp",
                                             name=f"ppk{half}_{th}_{i}")
                                   for i in range(8)]
                            for c in range(DC):
                                wt = wp.tile([P, 4 * P], BF16, tag="wt",
                                             name=f"wkt{half}_{th}_{c}")
                                nc.sync.dma_start(
                                    out=wt,
                                    in_=W["Wk"][c * P:(c + 1) * P,
                                                th * 4 * P:(th + 1) * 4 * P])
                                for t4 in range(4):
                                    for rb in range(2):
                                        nc.tensor.matmul(
                                            pss[t4 * 2 + rb],
                                            wt[:, t4 * P:(t4 + 1) * P],
                                            xtc[c][:, rb * (SH // 2):
                                                   (rb + 1) * (SH // 2)],
                                            start=(c == 0), stop=(c == DC - 1))
                            for t4 in range(4):
                                t = th * 4 + t4
                                for rb in range(2):
                                    kd = drain.tile([P, SH // 2], BF16,
                                                    tag="kd",
                                                    name=f"kd{half}_{th}_{t4}_{rb}")
                                    nc.vector.tensor_scalar(
                                        out=kd, in0=pss[t4 * 2 + rb],
                                        scalar1=vt["bk"][:, t:t + 1],
                                        scalar2=None, op0=ALU.add)
                                    nc.sync.dma_start(
                                        out=kt_full[t * P:(t + 1) * P,
                                                    half * SH + rb * (SH // 2):
                                                    half * SH + (rb + 1) * (SH // 2)],
                                        in_=kd)
                        DB = 512
                        for dv in range(D // DB):
                            pss = [psum.tile([P, DB], F32, tag="pp",
                                             name=f"ppv{half}_{dv}_{i}")
                                   for i in range(SCH)]
                            for c in range(DC):
                                wt = wp.tile([P, DB], BF16, tag="wt",
                                             name=f"wvt{half}_{dv}_{c}")
                                nc.sync.dma_start(
                                    out=wt,
                                    in_=W["Wv"][c * P:(c + 1) * P,
                                                dv * DB:(dv + 1) * DB])
                                for r8 in range(SCH):
                                    nc.tensor.matmul(
                                        pss[r8],
                                        xtc[c][:, r8 * P:(r8 + 1) * P],
                                        wt, start=(c == 0), stop=(c == DC - 1))
                            for r8 in range(SCH):
                                rt = half * SCH + r8
                                nc.vector.tensor_add(
                                    v_sb[:, rt, dv * DB:(dv + 1) * DB],
                                    pss[r8],
                                    bvb[:, dv * DB:(dv + 1) * DB])

                # ---- Q for own rows (after K/V so collectives start early);
                # t-outer so qt chunk 0 (attention pair 0) drains first ----
                wqt = []
                for c in range(DC):
                    wt = wp.tile([P, D], BF16, tag="wqt", name=f"wqt{c}")
                    nc.sync.dma_start(out=wt, in_=W["Wq"][c * P:(c + 1) * P, :])
                    wqt.append(wt)
                for t in range(DC):
                    ps = psum.tile([P, NQ], F32, tag="pp", name=f"pp_q{t}")
                    for c in range(DC):
                        nc.tensor.matmul(
                            ps, wqt[c][:, t * P:(t + 1) * P], xq[:, c, :],
                            start=(c == 0), stop=(c == DC - 1))
                    nc.vector.tensor_scalar(
                        out=qt[:, t, :], in0=ps,
                        scalar1=vt["bqs"][:, t:t + 1], scalar2=None,
                        op0=ALU.add)

            # ================= phase 2: attention =================
            attnp_cm = tc.tile_pool(name="attnp", bufs=1)
            attnp = attnp_cm.__enter__()
            # paired layout: even head on partitions 0-63, odd head on 64-127
            attn_t2 = attnp.tile([P, H // 2, NQ], BF16, tag="attn_t2")
            if kv_shard:
                for dv in range(2):
                    for sc in range(SC):
                        nc.sync.dma_start(
                            out=v_sb[:, sc, dv * DH:(dv + 1) * DH],
                            in_=v_all[dv][sc * P:(sc + 1) * P, :])
            # prefetch Wo (head-pair-stacked rows) during attention
            wop_cm = tc.tile_pool(name="wop", bufs=H // 2)
            wop = wop_cm.__enter__()
            wo_t = []
            for p8 in range(H // 2):
                wt = wop.tile([P, D], BF16, tag="wot", name=f"wot{p8}")
                nc.sync.dma_start(out=wt,
                                  in_=W["Wo"][p8 * P:(p8 + 1) * P, :])
                wo_t.append(wt)
            with ExitStack() as ph:
                kth_p = ph.enter_context(tc.tile_pool(name="kth",
                                                      bufs=kth_bufs))
                pt_p = ph.enter_context(tc.tile_pool(name="pt", bufs=pt_bufs))
                nrm = ph.enter_context(tc.tile_pool(name="nrm", bufs=2))
                sc_ps = ph.enter_context(tc.tile_pool(name="scp", bufs=sc_bufs,
                                                      space="PSUM"))
                at_ps = ph.enter_context(tc.tile_pool(name="atp", bufs=1,
                                                      space="PSUM"))
                dn_ps = ph.enter_context(tc.tile_pool(name="dnp", bufs=2,
                                                      space="PSUM"))
                bc_ps = ph.enter_context(tc.tile_pool(name="bcp", bufs=1,
                                                      space="PSUM"))

                def load_kth(hp):
                    h0, h1 = 2 * hp, 2 * hp + 1
                    kth = kth_p.tile([P, S], BF16, tag="kth",
                                     name=f"kth{hp}")
                    if kv_shard:
                        kh, f0 = divmod(h0 * HD, DH)
                        f1 = (h1 * HD) % DH
                        for r in range(NG):
                            nc.sync.dma_start(
                                out=kth[0:HD, r * SQ:(r + 1) * SQ],
                                in_=kt_all[kh][r * DH + f0:
                                               r * DH + f0 + HD, :])
                            nc.sync.dma_start(
                                out=kth[HD:2 * HD, r * SQ:(r + 1) * SQ],
                                in_=kt_all[kh][r * DH + f1:
                                               r * DH + f1 + HD, :])
                    else:
                        nc.sync.dma_start(out=kth[0:HD, :],
                                          in_=kt_full[h0 * HD:(h0 + 1) * HD, :])
                        nc.sync.dma_start(out=kth[HD:2 * HD, :],
                                          in_=kt_full[h1 * HD:(h1 + 1) * HD, :])
                    return kth

                def make_scores(hp, cp, kth):
                    sps = [sc_ps.tile([P, 2, NQ], F32, tag="sp",
                                      name=f"sp{hp}_{cp}_{j}")
                           for j in range(2)]
                    for j in range(2):
                        ck = cp * 2 + j
                        for hh, pol in ((0, 0), (1, HD)):
                            nc.tensor.matmul(
                                sps[hh][:, j, :],
                                kth[pol:pol + HD, ck * P:(ck + 1) * P],
                                qt[pol:pol + HD, hp, :],
                                start=True, stop=True)
                    return sps

                def normalize(hp, aps, dn):
                    rcps = []
                    for hh, dpart in ((0, 0), (1, 32)):
                        rcp = nrm.tile([1, NQ], BF16, tag="rcp",
                                       name=f"rcp{hp}_{hh}")
                        with nc.allow_low_precision(reason="bf16 bcast feed"):
                            nc.vector.reciprocal(out=rcp,
                                                 in_=dn[dpart:dpart + 1, :])
                        rcps.append(rcp)
                    bcp = bc_ps.tile([P, NQ], F32, tag="bcp",
                                     name=f"bcp{hp}")
                    nc.tensor.matmul(bcp[0:HD, :], ones_row[:, 0:HD], rcps[0],
                                     start=True, stop=True,
                                     tile_position=(0, 0))
                    nc.tensor.matmul(bcp[HD:P, :], ones_row[:, 0:HD], rcps[1],
                                     start=True, stop=True,
                                     tile_position=(0, 64))
                    bcs = nrm.tile([P, NQ], F32, tag="bcs", name=f"bcs{hp}")
                    nc.vector.tensor_copy(out=bcs, in_=bcp)
                    nc.vector.tensor_mul(attn_t2[:, hp, :], aps, bcs)

                # software-pipelined: scores(cp+1) issues ahead of attnV(cp)
                # so the in-order PE queue never blocks the ACT exp stream;
                # the previous pair's normalization is folded into this
                # pair's first iteration.
                pend = None   # (hp, aps, dn) awaiting normalization
                NPAIR = H // 2 if "attn" not in skip else 0
                kth_cur = load_kth(0) if NPAIR else None
                for hp in range(NPAIR):
                    h0, h1 = 2 * hp, 2 * hp + 1
                    aps = at_ps.tile([P, NQ], F32, tag="aps",
                                     name=f"aps{hp}")
                    dn = dn_ps.tile([33, NQ], F32, tag="dn", name=f"dn{hp}")
                    s_cur = make_scores(hp, 0, kth_cur)
                    kth_nxt = load_kth(hp + 1) if hp + 1 < NPAIR else None
                    for cp in range(SC // 2):
                        pts = []
                        for hh in range(2):
                            pt = pt_p.tile([P, 2, NQ], BF16, tag="pt",
                                           name=f"pt{hp}_{cp}_{hh}")
                            nc.scalar.activation(out=pt, in_=s_cur[hh],
                                                 func=AF.Exp)
                            pts.append(pt)
                        if cp == 0 and pend is not None:
                            # prev pair's normalization: must precede this
                            # pair's attnV in the PE queue (aps slot WAR)
                            normalize(*pend)
                            pend = None
                        if cp + 1 < SC // 2:
                            s_cur = make_scores(hp, cp + 1, kth_cur)
                        for j in range(2):
                            ck = cp * 2 + j
                            # h0 -> psum partitions 0-63, h1 -> 64-127
                            # (concurrent col groups); denominators to dn
                            nc.tensor.matmul(
                                aps[0:HD, :],
                                v_sb[:, ck, h0 * HD:(h0 + 1) * HD],
                                pts[0][:, j, :],
                                start=(ck == 0), stop=(ck == SC - 1),
                                tile_position=(0, 0))
                            nc.tensor.matmul(
                                aps[HD:P, :],
                                v_sb[:, ck, h1 * HD:(h1 + 1) * HD],
                                pts[1][:, j, :],
                                start=(ck == 0), stop=(ck == SC - 1),
                                tile_position=(0, 64))
                            nc.tensor.matmul(
                                dn[0:1, :], ones_col, pts[0][:, j, :],
                                start=(ck == 0), stop=(ck == SC - 1),
                                tile_position=(0, 0))
                            nc.tensor.matmul(
                                dn[32:33, :], ones_col, pts[1][:, j, :],
                                start=(ck == 0), stop=(ck == SC - 1),
                                tile_position=(0, 32))
                    pend = (hp, aps, dn)
                    kth_cur = kth_nxt
                if pend is not None:
                    normalize(*pend)

            # ---- out-proj (head pair stacked: K=128) + residual -> ln_in ---
            with ExitStack() as ph:
                odr = ph.enter_context(tc.tile_pool(name="odr", bufs=3))
                op_ps = ph.enter_context(tc.tile_pool(name="opp", bufs=8,
                                                      space="PSUM"))
                pss = [op_ps.tile([P, NQ], F32, tag="op", name=f"op_{i}")
                       for i in range(DC)]
                for p8 in range(H // 2):
                    for t in range(DC):
                        nc.tensor.matmul(
                            pss[t], wo_t[p8][:, t * P:(t + 1) * P],
                            attn_t2[:, p8, :],
                            start=(p8 == 0), stop=(p8 == H // 2 - 1))
                for t in range(DC):
                    oa = odr.tile([P, NQ], F32, tag="oa")
                    nc.vector.tensor_scalar(
                        out=oa, in0=pss[t], scalar1=vt["bo"][:, t:t + 1],
                        scalar2=None, op0=ALU.add)
                    nc.vector.tensor_add(ln_in[:, t, :], oa, xq[:, t, :])
            wop_cm.__exit__(None, None, None)  # free Wo tiles

            attnp_cm.__exit__(None, None, None)  # free attn_t2
            actp_cm.__exit__(None, None, None)   # free qt, xq, v_sb

            lnp_cm = tc.tile_pool(name="lnp", bufs=1)
            lnp = lnp_cm.__enter__()

            def layer_norm(ph, src, g, b, tag, out_dtype=BF16):
                """src [P, DC, NQ] bf16 feature-major -> LN'd tile from lnp."""
                st_ps = ph.enter_context(tc.tile_pool(name=f"st{tag}", bufs=1,
                                                      space="PSUM"))
                bb_ps = ph.enter_context(tc.tile_pool(name=f"bb{tag}", bufs=2,
                                                      space="PSUM"))
                sqp = ph.enter_context(tc.tile_pool(name=f"sq{tag}",
                                                    bufs=sq_bufs))
                row = ph.enter_context(tc.tile_pool(name=f"row{tag}", bufs=1))

                sm = st_ps.tile([1, NQ], F32, tag="sm")
                sq = st_ps.tile([1, NQ], F32, tag="sq")
                for c in range(DC):
                    nc.tensor.matmul(sm, ones_col, src[:, c, :],
                                     start=(c == 0), stop=(c == DC - 1))
                for c in range(DC):
                    x2 = sqp.tile([P, NQ], BF16, tag="x2")
                    # squares on ACT (idle here); sums run on PE
                    nc.scalar.activation(out=x2, in_=src[:, c, :],
                                         func=AF.Square)
                    nc.tensor.matmul(sq, ones_col, x2,
                                     start=(c == 0), stop=(c == DC - 1))
                mean = row.tile([1, NQ], F32, tag="mean")
                nc.scalar.mul(out=mean, in_=sm, mul=1.0 / D)
                msq = row.tile([1, NQ], F32, tag="msq")
                nc.scalar.mul(out=msq, in_=sq, mul=1.0 / D)
                var = row.tile([1, NQ], F32, tag="var")
                nc.vector.tensor_mul(var, mean, mean)
                nc.vector.tensor_sub(var, msq, var)
                sd = row.tile([1, NQ], F32, tag="sd")
                nc.scalar.activation(out=sd, in_=var, func=AF.Sqrt, bias=eps_t)
                rstd = row.tile([1, NQ], BF16, tag="rstd")
                with nc.allow_low_precision(reason="bf16 feed for bcast"):
                    nc.vector.reciprocal(out=rstd, in_=sd)
                shift = row.tile([1, NQ], BF16, tag="shift")   # -mean*rstd
                nc.vector.tensor_mul(shift, mean, rstd)
                nc.scalar.mul(out=shift, in_=shift, mul=-1.0)

                ab = bb_ps.tile([P, NQ], F32, tag="ab")
                nc.tensor.matmul(ab, ones_row, rstd, start=True, stop=True)
                a_b = sqp.tile([P, NQ], F32, tag="a_b")
                nc.vector.tensor_copy(out=a_b, in_=ab)
                bb = bb_ps.tile([P, NQ], F32, tag="ab")
                nc.tensor.matmul(bb, ones_row, shift, start=True, stop=True)
                b_b = sqp.tile([P, NQ], F32, tag="b_b")
                nc.vector.tensor_copy(out=b_b, in_=bb)

                out_t = lnp.tile([P, DC, NQ], out_dtype, tag=f"ln{tag}")
                for c in range(DC):
                    tmp = sqp.tile([P, NQ], F32, tag="tmp")
                    nc.vector.tensor_mul(tmp, src[:, c, :], a_b)
                    nc.vector.tensor_add(tmp, tmp, b_b)
                    # gamma/beta via ACT (per-partition scale+bias), off DVE
                    nc.scalar.activation(
                        out=out_t[:, c, :], in_=tmp, func=AF.Identity,
                        scale=g[:, c:c + 1], bias=b[:, c:c + 1])
                return out_t

            # ================= phase 3: LN1, FFN, LN2 =================
            with ExitStack() as ph:
                ln1 = layer_norm(ph, ln_in, vt["g1"], vt["b1n"], "1")

            with ExitStack() as ph:
                ffn = ph.enter_context(tc.tile_pool(name="ffn", bufs=1))
                wfp = ph.enter_context(tc.tile_pool(name="wfp", bufs=wfp_bufs))
                fdr = ph.enter_context(tc.tile_pool(name="fdr", bufs=fdr_bufs))
                f_ps = ph.enter_context(tc.tile_pool(name="fps", bufs=fps_bufs,
                                                     space="PSUM"))
                h1 = ffn.tile([P, FC, NQ], BF16, tag="h1")
                for pg in (() if "ffn" in skip else range(FC // 8)):
                    pss = [f_ps.tile([P, NQ], F32, tag="fp",
                                     name=f"fp1_{pg}_{i}") for i in range(8)]
                    for c in range(DC):
                        wt = wfp.tile([P, 8 * P], BF16, tag="w1t")
                        nc.sync.dma_start(
                            out=wt,
                            in_=W["W1"][c * P:(c + 1) * P,
                                        pg * 8 * P:(pg + 1) * 8 * P])
                        for t8 in range(8):
                            nc.tensor.matmul(
                                pss[t8], wt[:, t8 * P:(t8 + 1) * P],
                                ln1[:, c, :],
                                start=(c == 0), stop=(c == DC - 1))
                    for t8 in range(8):
                        t = pg * 8 + t8
                        nc.scalar.activation(
                            out=h1[:, t, :], in_=pss[t8], func=AF.Gelu,
                            bias=vt["bf1"][:, t:t + 1])

                ln2_in = midp.tile([P, DC, NQ], BF16, tag="ln2_in")
                if "ffn" in skip:
                    for t in range(DC):
                        nc.vector.tensor_copy(out=ln2_in[:, t, :],
                                              in_=ln1[:, t, :])
                else:
                    pss = [f_ps.tile([P, NQ], F32, tag="fp", name=f"fp2_{i}")
                           for i in range(DC)]
                    for c in range(FC):
                        wt = wfp.tile([P, D], BF16, tag="w2t")
                        nc.sync.dma_start(out=wt,
                                          in_=W["W2"][c * P:(c + 1) * P, :])
                        for t in range(DC):
                            nc.tensor.matmul(
                                pss[t], wt[:, t * P:(t + 1) * P],
                                h1[:, c, :],
                                start=(c == 0), stop=(c == FC - 1))
                    for t in range(DC):
                        fo = fdr.tile([P, NQ], F32, tag="fo")
                        nc.vector.tensor_scalar(
                            out=fo, in0=pss[t], scalar1=vt["bf2"][:, t:t + 1],
                            scalar2=None, op0=ALU.add)
                        nc.vector.tensor_add(ln2_in[:, t, :], fo, ln1[:, t, :])

            with ExitStack() as ph:
                y_out = layer_norm(ph, ln2_in, vt["g2"], vt["b2n"], "2",
                                   out_dtype=F32)
                yv = yT.ap().rearrange("(t p) q -> p t q", p=P)
                for c in range(DC):
                    nc.sync.dma_start(out=yv[:, c, :], in_=y_out[:, c, :])
            lnp_cm.__exit__(None, None, None)
            midp_cm.__exit__(None, None, None)

    if compile:
        nc.compile()
    return nc


# ---------------- host-side sharding / gather ----------------
import numpy as np
import ml_dtypes

BF16_NP = ml_dtypes.bfloat16
B, S, D, H = 2, 2048, 1024, 16
HD = D // H
DFF = 4 * D
N_CORES = 8
CPB = N_CORES // B           # cores per batch element
SQ = S // CPB                # query rows per core

_nc = None


def _get_nc():
    global _nc
    if _nc is None:
        _nc = build(S=S, SQ=SQ, D=D, H=H, DFF=DFF, n_cores=N_CORES)
    return _nc


def _make_in_maps(inputs):
    x = np.ascontiguousarray(inputs["x"], dtype=np.float32)
    scale = np.float32(1.0 / np.sqrt(HD))
    bf = lambda a: np.ascontiguousarray(np.asarray(a, np.float32)).astype(BF16_NP)
    f32 = lambda a: np.ascontiguousarray(inputs[a], np.float32)
    shared = {
        "Wq": bf(np.asarray(inputs["Wq"], np.float32) * scale),
        "Wk": bf(inputs["Wk"]),
        "Wv": bf(inputs["Wv"]),
        "Wo": bf(inputs["Wo"]),
        "W1": bf(inputs["W1"]),
        "W2": bf(inputs["W2"]),
        "bqs": np.ascontiguousarray(inputs["bq"], np.float32) * scale,
        "bk": f32("bk"), "bv": f32("bv"), "bo": f32("bo"),
        "bf1": f32("bf1"), "bf2": f32("bf2"),
        "g1": f32("g1"), "b1n": f32("b1n"), "g2": f32("g2"), "b2n": f32("b2n"),
        "ones_d": np.ones(128, BF16_NP),
    }
    xT = np.ascontiguousarray(x.transpose(0, 2, 1)).astype(BF16_NP)  # [B,D,S]
    in_maps = []
    for c in range(N_CORES):
        b, q0 = c // CPB, (c % CPB) * SQ
        m = dict(shared)
        m["xqT"] = np.ascontiguousarray(xT[b][:, q0:q0 + SQ])
        in_maps.append(m)
    return in_maps


def kernel(**inputs):
    from concourse.bass_utils import run_bass_kernel_spmd
    nc = _get_nc()
    in_maps = _make_in_maps(inputs)
    res = run_bass_kernel_spmd(nc, in_maps, core_ids=list(range(N_CORES)))
    y = np.empty((B, S, D), dtype=np.float32)
    for c in range(N_CORES):
        b, q0 = c // CPB, (c % CPB) * SQ
        y[b, q0:q0 + SQ, :] = res.results[c]["yT"].T
    return y
